# revision 13
# baseline (speedup 1.0000x reference)
"""Causal multi-head attention (qkv proj + attention + out proj) on 8 TRN2 cores.

Problem: x[2,2048,512] -> qkv proj (w_qkv [512,1536]) -> 8 heads x 64 dim causal
attention -> out proj (w_out [512,512] + b_out). Key-padding mask is all-ones
per the problem spec, so only the causal mask is applied.

Sharding: data-parallel over batch (2) x tensor-parallel over heads (4 groups
of 2 heads).  Core c handles batch c//4 and heads {2*(c%4), 2*(c%4)+1}.  Each
core computes its 2 heads' partial out-projection [N, DIM]; the host sums the
4 partials per batch and adds b_out (the unshard step for TP-partial outputs).

Per-core kernel:
  - Both heads processed as one 128-wide unit wherever possible: qkv
    projections produce qT2/kT2/vT2 [128, N] (heads stacked on partitions,
    M=128 matmuls), and the out-projection contracts K=128 over both heads in
    a single matmul per row tile.
  - Attention per head uses partition-base-offset slices of qT2/kT2 in
    fp32r (full rate at free dim >= 256), transposed-probs orientation:
    dotsT[j,i] = k_j . q_i per (i-block 512, j-chunk 128); exp on ScalarE with
    no max subtraction (logits bounded, softmax shift-invariant); causal mask
    multiplies only the 128x128 diagonal sub-block by a fixed triangular
    mask; P@V appends a shared ones-column to V so row sums land in PSUM free;
    normalization via fast approximate reciprocal.
  - Emission is one software-pipelined stream: attention chunks carry the
    next block's DMA/transpose/projection ops and the previous block's
    out-projection as spread filler; P@V lags its dots by one chunk so the
    TensorE never waits on ScalarE's exp.
"""

import numpy as np

B, N, DIM = 2, 2048, 512
HEADS, DH = 8, 64
SCALE = DH ** -0.5
NT = N // 128      # 16 row tiles
NB = N // 512      # 4 blocks
CC = DIM // 128    # 4 contraction chunks
NCORES = 8

_cache = {}


def _build():
    import concourse.bass as bass
    import concourse.mybir as mybir
    import concourse.tile as tile
    from concourse import bacc
    from contextlib import ExitStack

    F32 = mybir.dt.float32
    F32R = mybir.dt.float32r
    BF16 = mybir.dt.bfloat16
    Exp = mybir.ActivationFunctionType.Exp

    nc = bacc.Bacc()
    x_d = nc.declare_dram_parameter("x", [N, DIM], F32, isOutput=False).ap()
    # qkv weights feed fp32r matmuls -> declare fp32r so every producer in the
    # chain is a legal fp32r source for the BIR verifier.
    wq_d = nc.declare_dram_parameter("wq", [DIM, 128], F32R, isOutput=False).ap()
    wk_d = nc.declare_dram_parameter("wk", [DIM, 128], F32R, isOutput=False).ap()
    wv_d = nc.declare_dram_parameter("wv", [DIM, 128], F32R, isOutput=False).ap()
    wo_d = nc.declare_dram_parameter("wo", [128, DIM], F32, isOutput=False).ap()
    out_d = nc.declare_dram_parameter("out", [N, DIM], BF16, isOutput=True).ap()

    with tile.TileContext(nc) as tc:
        with ExitStack() as ctx:
            persist = ctx.enter_context(tc.tile_pool(name="persist", bufs=1))

            # --- constants ---
            id_f = persist.tile([128, 128], F32, tag="idf")
            nc.gpsimd.memset(id_f, 0.0)
            nc.gpsimd.affine_select(
                out=id_f, in_=id_f, compare_op=mybir.AluOpType.not_equal,
                fill=1.0, base=0, pattern=[[-1, 128]], channel_multiplier=1)
            id_b = persist.tile([128, 128], BF16, tag="idb")
            nc.gpsimd.memset(id_b, 0.0)
            nc.gpsimd.affine_select(
                out=id_b, in_=id_b, compare_op=mybir.AluOpType.not_equal,
                fill=1.0, base=0, pattern=[[-1, 128]], channel_multiplier=1)
            # tri[p, x] = 1.0 if x >= p else 0.0 (keep i >= j on the diagonal)
            tri = persist.tile([128, 128], BF16, tag="tri")
            nc.gpsimd.memset(tri, 1.0)
            nc.gpsimd.affine_select(
                out=tri, in_=tri, compare_op=mybir.AluOpType.is_ge,
                fill=0.0, base=0, pattern=[[1, 128]], channel_multiplier=-1)

            # --- weights ---
            wq_sb = persist.tile([128, CC, 128], F32R, tag="wq")
            nc.sync.dma_start(out=wq_sb, in_=wq_d.rearrange("(c p) d -> p c d", p=128))
            wk_sb = persist.tile([128, CC, 128], F32R, tag="wk")
            nc.sync.dma_start(out=wk_sb, in_=wk_d.rearrange("(c p) d -> p c d", p=128))
            wv_sb = persist.tile([128, CC, 128], F32R, tag="wv")
            nc.sync.dma_start(out=wv_sb, in_=wv_d.rearrange("(c p) d -> p c d", p=128))
            wo_sb = persist.tile([128, DIM], F32, tag="wo32")
            nc.sync.dma_start(out=wo_sb, in_=wo_d)
            wo_bf = persist.tile([128, DIM], BF16, tag="wobf")
            nc.vector.tensor_copy(out=wo_bf, in_=wo_sb)

            # --- persistent activations (both heads stacked) ---
            xT = persist.tile([128, CC, N], F32R, tag="xT")
            qT2 = persist.tile([128, N], F32R, tag="qT2")
            kT2 = persist.tile([128, N], F32R, tag="kT2")
            # vo: [v_h0 (0:64) | ones (64) | v_h1 (65:129)] -- ones shared.
            # av rhs for h0 = vo[:, t, 0:65] (sum in col 64); for h1 =
            # vo[:, t, 64:129] (sum in col 0).
            vo = persist.tile([128, NT, 129], BF16, tag="vo")
            nc.gpsimd.memset(vo, 1.0)
            ohT2 = persist.tile([128, N], BF16, tag="ohT2")

            pools = [
                tc.tile_pool(name="xs", bufs=4),
                tc.tile_pool(name="vts", bufs=2),
                tc.tile_pool(name="probs", bufs=4),
                tc.tile_pool(name="small", bufs=8),
                tc.tile_pool(name="stage", bufs=3),
                tc.tile_pool(name="proj", bufs=2, space="PSUM"),   # transposes+qkv
                tc.tile_pool(name="pdots", bufs=2, space="PSUM"),  # dots + outproj
                tc.tile_pool(name="pav", bufs=1, space="PSUM"),    # 4 tagged av banks
            ]
            (xs_pool, vt_pool, pr_pool, sm_pool, st_pool,
             pj_pool, dt_pool, av_pool) = [
                ctx.enter_context(p) for p in pools]

            # PE consumes the gpsimd-built constants once, so later transpose
            # matmuls carry at most one semaphore wait (the LDWEIGHTS slot
            # only fits one) -- their Pool dependency is then transitive.
            pwarm = pj_pool.tile([128, 128], F32, tag="pj", name="pwarm")
            nc.tensor.transpose(out=pwarm, in_=id_f, identity=id_f)
            warm_sb = sm_pool.tile([128, 128], F32, tag="warm", name="warm")
            nc.vector.tensor_copy(out=warm_sb, in_=pwarm)

            def xT_ops(g):
                """Closures loading + transposing x block g into xT."""
                ops = []
                state = {}

                def mk_t(t):
                    def dma():
                        xs = xs_pool.tile([128, DIM], F32, tag="xs")
                        nc.sync.dma_start(
                            out=xs, in_=x_d[t * 128:(t + 1) * 128, :])
                        state[t] = xs

                    def mk_tr(c):
                        def f():
                            if c == 0:
                                state[(t, "px")] = pj_pool.tile(
                                    [128, 4, 128], F32, tag="pj", name="px")
                            nc.tensor.transpose(
                                out=state[(t, "px")][:, c, :],
                                in_=state[t][:, c * 128:(c + 1) * 128],
                                identity=id_f)
                        return f

                    def cp():
                        nc.vector.tensor_copy(
                            out=xT[:, :, t * 128:(t + 1) * 128],
                            in_=state.pop((t, "px")))
                        state.pop(t)
                    return [dma] + [mk_tr(c) for c in range(CC)] + [cp]

                for t in range(4 * g, 4 * g + 4):
                    ops.extend(mk_t(t))
                return ops

            def qkv_ops(g):
                """Closures projecting q/k/v (both heads at once) for block g."""
                ops = []
                state = {}

                def mk_mm(key, wsb, c):
                    def f():
                        if c == 0:
                            state[key] = pj_pool.tile(
                                [128, 512], F32, tag="pj", name=f"ps_{key}")
                        nc.tensor.matmul(
                            out=state[key],
                            lhsT=wsb[:, c, :],
                            rhs=xT[:, c, g * 512:(g + 1) * 512],
                            start=(c == 0), stop=(c == CC - 1))
                    return f

                def mk_cp(key, dst):
                    def f():
                        nc.vector.tensor_copy(
                            out=dst[:, g * 512:(g + 1) * 512],
                            in_=state.pop(key))
                    return f

                for key, (wsb, dst) in enumerate(
                        ((wq_sb, qT2), (wk_sb, kT2))):
                    for c in range(CC):
                        ops.append(mk_mm(key, wsb, c))
                    ops.append(mk_cp(key, dst))
                for c in range(CC):
                    ops.append(mk_mm("v", wv_sb, c))

                def cp_v():
                    vts = vt_pool.tile([128, 512], F32, tag="vts")
                    nc.vector.tensor_copy(out=vts, in_=state.pop("v"))
                    state["vts"] = vts
                ops.append(cp_v)

                def mk_tr(i):
                    def f():
                        if i == 0:
                            state["pv"] = pj_pool.tile(
                                [128, 4, 128], F32, tag="pj", name="pv")
                        nc.tensor.transpose(
                            out=state["pv"][:, i, :],
                            in_=state["vts"][:, i * 128:(i + 1) * 128],
                            identity=id_f)
                    return f
                for i in range(4):
                    ops.append(mk_tr(i))

                def cp_vo0():
                    nc.vector.tensor_copy(
                        out=vo[:, 4 * g:4 * g + 4, 0:64],
                        in_=state["pv"][:, :, 0:64])

                def cp_vo1():
                    nc.vector.tensor_copy(
                        out=vo[:, 4 * g:4 * g + 4, 65:129],
                        in_=state.pop("pv")[:, :, 64:128])
                    state.pop("vts", None)
                ops.extend([cp_vo0, cp_vo1])
                return ops

            def outproj_ops(g):
                """Closures for the block-g out-projection (heads fused, K=128)."""
                ops = []
                state = {}

                def mk(s):
                    t = g * 4 + s

                    def mm():
                        state[s] = dt_pool.tile(
                            [128, DIM], F32, tag="dots", name="pp")
                        nc.tensor.matmul(
                            out=state[s], lhsT=ohT2[:, t * 128:(t + 1) * 128],
                            rhs=wo_bf, start=True, stop=True)

                    def cp():
                        st = st_pool.tile([128, DIM], BF16, tag="st")
                        nc.vector.tensor_copy(out=st, in_=state.pop(s))
                        nc.sync.dma_start(
                            out=out_d[t * 128:(t + 1) * 128, :], in_=st)
                    return [mm, cp]

                for s in range(4):
                    ops.extend(mk(s))
                return ops

            def emit_attn(h, g, oh_g, spread=()):
                """Attention for head h over i-block g; fills oh_g columns."""
                spread = list(spread)
                hb = h * 64
                qTh = qT2[hb:hb + 64, :]
                kTh = kT2[hb:hb + 64, :]
                sum_col = 64 if h == 0 else 0
                v_lo = 0 if h == 0 else 64
                av = [av_pool.tile([128, 65], F32, tag=f"av{s}", name=f"av{s}")
                      for s in range(4)]
                nch = 4 * g + 4
                per = -(-len(spread) // nch) if spread else 0
                pend = None  # (chunk index, probs tile) whose P@V is deferred

                def emit_av(pc, ppb):
                    pr = pc - 4 * g
                    for s in range(max(pr, 0), 4):
                        nc.tensor.matmul(
                            out=av[s],
                            lhsT=ppb[:, s * 128:(s + 1) * 128],
                            rhs=vo[:, pc, v_lo:v_lo + 65],
                            start=(pc == 0), stop=(pc == 4 * g + s))

                for c in range(nch):
                    r = c - 4 * g
                    lo = 128 * r if r > 0 else 0
                    dp = dt_pool.tile([128, 512], F32, tag="dots", name="dp")
                    nc.tensor.matmul(
                        out=dp[:, lo:512],
                        lhsT=kTh[:, c * 128:(c + 1) * 128],
                        rhs=qTh[:, g * 512 + lo:(g + 1) * 512],
                        start=True, stop=True)
                    pb = pr_pool.tile([128, 512], BF16, tag="probs", name="pb")
                    nc.scalar.activation(out=pb[:, lo:512], in_=dp[:, lo:512],
                                         func=Exp, scale=SCALE)
                    if r >= 0:
                        nc.vector.tensor_mul(
                            pb[:, lo:lo + 128], pb[:, lo:lo + 128], tri)
                    if pend is not None:
                        emit_av(*pend)
                    pend = (c, pb)
                    for _ in range(per):
                        if spread:
                            spread.pop(0)()
                emit_av(*pend)
                for s in range(4):
                    rec = sm_pool.tile([128, 1], F32, tag="rec", name="rec")
                    nc.vector.reciprocal_approx_fast(
                        out=rec, in_=av[s][:, sum_col:sum_col + 1])
                    osl = av[s][:, 0:64] if h == 0 else av[s][:, 1:65]
                    nc.vector.tensor_scalar_mul(
                        oh_g[:, s, hb:hb + 64], osl, rec)
                for op in spread:
                    op()

            def ohT_flush(g, oh_g):
                """Transpose the block's stacked head outputs into ohT2."""
                for s in range(4):
                    pt = pj_pool.tile([128, 128], BF16, tag="pj", name="pt")
                    nc.tensor.transpose(
                        out=pt, in_=oh_g[:, s, :], identity=id_b)
                    t = g * 4 + s
                    nc.vector.tensor_copy(
                        out=ohT2[:, t * 128:(t + 1) * 128], in_=pt)

            # --- software-pipelined emission ---
            for op in xT_ops(0) + qkv_ops(0):
                op()
            for g in range(NB):
                oh_g = sm_pool.tile([128, 4, 128], BF16, tag="ohg", name="ohg",
                                    bufs=2)
                sp0 = outproj_ops(g - 1) if g > 0 else []
                emit_attn(0, g, oh_g, spread=sp0)
                nxt = xT_ops(g + 1) + qkv_ops(g + 1) if g + 1 < NB else []
                emit_attn(1, g, oh_g, spread=nxt)
                ohT_flush(g, oh_g)
            for op in outproj_ops(NB - 1):
                op()
    nc.compile()
    return nc


def _get_nc():
    if "nc" not in _cache:
        _cache["nc"] = _build()
    return _cache["nc"]


def _in_maps(x, w_qkv, w_out):
    maps = []
    for c in range(NCORES):
        b = c // 4
        h0 = 2 * (c % 4)
        cols = slice(h0 * DH, (h0 + 2) * DH)  # 128 contiguous head cols
        maps.append({
            "x": np.ascontiguousarray(x[b]),
            "wq": np.ascontiguousarray(w_qkv[:, 0:512][:, cols]),
            "wk": np.ascontiguousarray(w_qkv[:, 512:1024][:, cols]),
            "wv": np.ascontiguousarray(w_qkv[:, 1024:1536][:, cols]),
            "wo": np.ascontiguousarray(w_out[cols, :]),
        })
    return maps


def _combine(results, b_out):
    out = np.zeros((B, N, DIM), np.float32)
    for c in range(NCORES):
        out[c // 4] += np.asarray(results[c]["out"], dtype=np.float32)
    out += b_out.astype(np.float32)
    return out


def kernel(**inputs):
    x = np.asarray(inputs["x"], dtype=np.float32)
    w_qkv = np.asarray(inputs["w_qkv"], dtype=np.float32)
    w_out = np.asarray(inputs["w_out"], dtype=np.float32)
    b_out = np.asarray(inputs["b_out"], dtype=np.float32)
    # inputs["mask"] is all-ones per the problem spec (key padding no-op).
    from concourse.bass_utils import run_bass_kernel_spmd
    nc = _get_nc()
    res = run_bass_kernel_spmd(nc, _in_maps(x, w_qkv, w_out), list(range(NCORES)))
    return _combine(res.results, b_out)


# revision 19
# speedup vs baseline: 1.1001x; 1.1001x over previous
"""Causal multi-head attention (qkv proj + attention + out proj) on 8 TRN2 cores.

Problem: x[2,2048,512] -> qkv proj (w_qkv [512,1536]) -> 8 heads x 64 dim causal
attention -> out proj (w_out [512,512] + b_out). Key-padding mask is all-ones
per the problem spec, so only the causal mask is applied.

Sharding: data-parallel over batch (2) x tensor-parallel over heads (4 groups
of 2 heads).  Core c handles batch c//4 and heads {2*(c%4), 2*(c%4)+1}.  Each
core computes its 2 heads' partial out-projection [N, DIM]; the host sums the
4 partials per batch and adds b_out (the unshard step for TP-partial outputs).

Per-core kernel:
  - Both heads processed as one 128-wide unit wherever possible: qkv
    projections produce qT2/kT2/vT2 [128, N] (heads stacked on partitions,
    M=128 matmuls), and the out-projection contracts K=128 over both heads in
    a single matmul per row tile.
  - Attention per head uses partition-base-offset slices of qT2/kT2 in
    fp32r (full rate at free dim >= 256), transposed-probs orientation:
    dotsT[j,i] = k_j . q_i per (i-block 512, j-chunk 128); exp on ScalarE with
    no max subtraction (logits bounded, softmax shift-invariant); causal mask
    multiplies only the 128x128 diagonal sub-block by a fixed triangular
    mask; P@V appends a shared ones-column to V so row sums land in PSUM free;
    normalization via fast approximate reciprocal.
  - Emission is one software-pipelined stream: attention chunks carry the
    next block's DMA/transpose/projection ops and the previous block's
    out-projection as spread filler; P@V lags its dots by one chunk so the
    TensorE never waits on ScalarE's exp.
"""

import numpy as np

B, N, DIM = 2, 2048, 512
HEADS, DH = 8, 64
SCALE = DH ** -0.5
NT = N // 128      # 16 row tiles
NB = N // 512      # 4 blocks
CC = DIM // 128    # 4 contraction chunks
NCORES = 8

_cache = {}


def _build():
    import concourse.bass as bass
    import concourse.mybir as mybir
    import concourse.tile as tile
    from concourse import bacc
    from contextlib import ExitStack

    F32 = mybir.dt.float32
    F32R = mybir.dt.float32r
    BF16 = mybir.dt.bfloat16
    Exp = mybir.ActivationFunctionType.Exp

    nc = bacc.Bacc()
    x_d = nc.declare_dram_parameter("x", [N, DIM], F32, isOutput=False).ap()
    # qkv weights feed fp32r matmuls -> declare fp32r so every producer in the
    # chain is a legal fp32r source for the BIR verifier.
    wq_d = nc.declare_dram_parameter("wq", [DIM, 128], F32R, isOutput=False).ap()
    wk_d = nc.declare_dram_parameter("wk", [DIM, 128], F32R, isOutput=False).ap()
    wv_d = nc.declare_dram_parameter("wv", [DIM, 128], F32R, isOutput=False).ap()
    wo_d = nc.declare_dram_parameter("wo", [128, DIM], F32, isOutput=False).ap()
    out_d = nc.declare_dram_parameter("out", [N, DIM], BF16, isOutput=True).ap()

    with tile.TileContext(nc) as tc:
        with ExitStack() as ctx:
            persist = ctx.enter_context(tc.tile_pool(name="persist", bufs=1))

            # --- constants ---
            id_f = persist.tile([128, 128], F32, tag="idf")
            nc.vector.memset(id_f, 0.0)
            nc.gpsimd.affine_select(
                out=id_f, in_=id_f, compare_op=mybir.AluOpType.not_equal,
                fill=1.0, base=0, pattern=[[-1, 128]], channel_multiplier=1)
            warm_sb = persist.tile([128, 128], F32, tag="warm")
            id_b = persist.tile([128, 128], BF16, tag="idb")
            nc.vector.memset(id_b, 0.0)
            nc.gpsimd.affine_select(
                out=id_b, in_=id_b, compare_op=mybir.AluOpType.not_equal,
                fill=1.0, base=0, pattern=[[-1, 128]], channel_multiplier=1)
            # tri[p, x] = 1.0 if x >= p else 0.0 (keep i >= j on the diagonal)
            tri = persist.tile([128, 128], BF16, tag="tri")
            nc.vector.memset(tri, 1.0)
            nc.gpsimd.affine_select(
                out=tri, in_=tri, compare_op=mybir.AluOpType.is_ge,
                fill=0.0, base=0, pattern=[[1, 128]], channel_multiplier=-1)

            # --- weights (DMAs emitted after block-0 x loads; see below) ---
            wq_sb = persist.tile([128, CC, 128], F32R, tag="wq")
            wk_sb = persist.tile([128, CC, 128], F32R, tag="wk")
            wv_sb = persist.tile([128, CC, 128], F32R, tag="wv")
            wo_sb = persist.tile([128, DIM], F32, tag="wo32")
            wo_bf = persist.tile([128, DIM], BF16, tag="wobf")

            def emit_weight_dmas():
                nc.sync.dma_start(
                    out=wq_sb, in_=wq_d.rearrange("(c p) d -> p c d", p=128))
                nc.sync.dma_start(
                    out=wk_sb, in_=wk_d.rearrange("(c p) d -> p c d", p=128))
                nc.sync.dma_start(
                    out=wv_sb, in_=wv_d.rearrange("(c p) d -> p c d", p=128))

            def emit_wo_dma():
                nc.sync.dma_start(out=wo_sb, in_=wo_d)
                nc.vector.tensor_copy(out=wo_bf, in_=wo_sb)

            # --- persistent activations (both heads stacked) ---
            xT = persist.tile([128, CC, N], F32R, tag="xT")
            qT2 = persist.tile([128, N], F32R, tag="qT2")
            kT2 = persist.tile([128, N], F32R, tag="kT2")
            # vo: [v_h0 (0:64) | ones (64) | v_h1 (65:129)] -- ones shared.
            # av rhs for h0 = vo[:, t, 0:65] (sum in col 64); for h1 =
            # vo[:, t, 64:129] (sum in col 0).
            vo = persist.tile([128, NT, 129], BF16, tag="vo")
            nc.vector.memset(vo, 1.0)
            ohT2 = persist.tile([128, N], BF16, tag="ohT2")

            pools = [
                tc.tile_pool(name="xs", bufs=6),
                tc.tile_pool(name="vts", bufs=2),
                tc.tile_pool(name="probs", bufs=8),
                tc.tile_pool(name="small", bufs=8),
                tc.tile_pool(name="stage", bufs=3),
                tc.tile_pool(name="proj", bufs=2, space="PSUM"),   # transposes+qkv
                tc.tile_pool(name="pdots", bufs=2, space="PSUM"),  # dots + outproj
                tc.tile_pool(name="pav", bufs=1, space="PSUM"),    # 4 tagged av banks
            ]
            (xs_pool, vt_pool, pr_pool, sm_pool, st_pool,
             pj_pool, dt_pool, av_pool) = [
                ctx.enter_context(p) for p in pools]

            # PE consumes id_f early so x transposes only wait on their DMA.
            pwarm = pj_pool.tile([128, 128], F32, tag="pj", name="pwarm")
            nc.tensor.transpose(out=pwarm, in_=id_f, identity=id_f)
            nc.vector.tensor_copy(out=warm_sb, in_=pwarm)

            def xT_ops(g):
                """Closures loading + transposing x block g into xT."""
                ops = []
                state = {}

                def mk_t(t):
                    def dma():
                        xs = xs_pool.tile([128, DIM], F32, tag="xs")
                        nc.sync.dma_start(
                            out=xs, in_=x_d[t * 128:(t + 1) * 128, :])
                        state[t] = xs

                    def mk_tr(c):
                        def f():
                            if c == 0:
                                state[(t, "px")] = pj_pool.tile(
                                    [128, 4, 128], F32, tag="pj", name="px")
                            nc.tensor.transpose(
                                out=state[(t, "px")][:, c, :],
                                in_=state[t][:, c * 128:(c + 1) * 128],
                                identity=id_f)
                        return f

                    def cp():
                        nc.vector.tensor_copy(
                            out=xT[:, :, t * 128:(t + 1) * 128],
                            in_=state.pop((t, "px")))
                        state.pop(t)
                    return [dma] + [mk_tr(c) for c in range(CC)] + [cp]

                for t in range(4 * g, 4 * g + 4):
                    ops.extend(mk_t(t))
                return ops

            def qkv_ops(g):
                """Closures projecting q/k/v (both heads at once) for block g."""
                ops = []
                state = {}

                def mk_mm(key, wsb, c):
                    def f():
                        if c == 0:
                            state[key] = pj_pool.tile(
                                [128, 512], F32, tag="pj", name=f"ps_{key}")
                        nc.tensor.matmul(
                            out=state[key],
                            lhsT=wsb[:, c, :],
                            rhs=xT[:, c, g * 512:(g + 1) * 512],
                            start=(c == 0), stop=(c == CC - 1))
                    return f

                def mk_cp(key, dst):
                    def f():
                        nc.vector.tensor_copy(
                            out=dst[:, g * 512:(g + 1) * 512],
                            in_=state.pop(key))
                    return f

                for key, (wsb, dst) in enumerate(
                        ((wq_sb, qT2), (wk_sb, kT2))):
                    for c in range(CC):
                        ops.append(mk_mm(key, wsb, c))
                    ops.append(mk_cp(key, dst))
                for c in range(CC):
                    ops.append(mk_mm("v", wv_sb, c))

                def cp_v():
                    vts = vt_pool.tile([128, 512], F32, tag="vts")
                    nc.vector.tensor_copy(out=vts, in_=state.pop("v"))
                    state["vts"] = vts
                ops.append(cp_v)

                def mk_tr(i):
                    def f():
                        if i == 0:
                            state["pv"] = pj_pool.tile(
                                [128, 4, 128], F32, tag="pj", name="pv")
                        nc.tensor.transpose(
                            out=state["pv"][:, i, :],
                            in_=state["vts"][:, i * 128:(i + 1) * 128],
                            identity=id_f)
                    return f
                for i in range(4):
                    ops.append(mk_tr(i))

                def cp_vo0():
                    nc.vector.tensor_copy(
                        out=vo[:, 4 * g:4 * g + 4, 0:64],
                        in_=state["pv"][:, :, 0:64])

                def cp_vo1():
                    nc.vector.tensor_copy(
                        out=vo[:, 4 * g:4 * g + 4, 65:129],
                        in_=state.pop("pv")[:, :, 64:128])
                    state.pop("vts", None)
                ops.extend([cp_vo0, cp_vo1])
                return ops

            def outproj_ops(g):
                """Closures for the block-g out-projection (heads fused, K=128)."""
                ops = []
                state = {}

                def mk(s):
                    t = g * 4 + s

                    def mm():
                        state[s] = pj_pool.tile(
                            [128, DIM], F32, tag="pj", name="pp")
                        nc.tensor.matmul(
                            out=state[s], lhsT=ohT2[:, t * 128:(t + 1) * 128],
                            rhs=wo_bf, start=True, stop=True)

                    def cp():
                        st = st_pool.tile([128, DIM], BF16, tag="st")
                        nc.vector.tensor_copy(out=st, in_=state.pop(s))
                        nc.sync.dma_start(
                            out=out_d[t * 128:(t + 1) * 128, :], in_=st)
                    return [mm, cp]

                for s in range(4):
                    ops.extend(mk(s))
                return ops

            def emit_attn(h, g, oh_g, spread=(), tail=False):
                """Attention for head h over i-block g; fills oh_g columns."""
                spread = list(spread)
                hb = h * 64
                qTh = qT2[hb:hb + 64, :]
                kTh = kT2[hb:hb + 64, :]
                sum_col = 64 if h == 0 else 0
                v_lo = 0 if h == 0 else 64
                av = [av_pool.tile([128, 65], F32, tag=f"av{s}", name=f"av{s}")
                      for s in range(4)]
                nch = 4 * g + 4
                per = -(-(2 * len(spread)) // nch) if spread else 0
                pend = []  # (chunk index, probs tile) whose P@V is deferred

                def emit_norm(sb):
                    rec = sm_pool.tile([128, 1], F32, tag="rec", name="rec")
                    nc.vector.reciprocal_approx_fast(
                        out=rec, in_=av[sb][:, sum_col:sum_col + 1])
                    osl = av[sb][:, 0:64] if h == 0 else av[sb][:, 1:65]
                    nc.vector.tensor_scalar_mul(
                        oh_g[:, sb, hb:hb + 64], osl, rec)
                    if tail:
                        t = g * 4 + sb
                        pt = pj_pool.tile([128, 128], BF16, tag="pj", name="pt")
                        nc.tensor.transpose(
                            out=pt, in_=oh_g[:, sb, :], identity=id_b)
                        nc.vector.tensor_copy(
                            out=ohT2[:, t * 128:(t + 1) * 128], in_=pt)
                        pp = pj_pool.tile([128, DIM], F32, tag="pj", name="pp")
                        nc.tensor.matmul(
                            out=pp, lhsT=ohT2[:, t * 128:(t + 1) * 128],
                            rhs=wo_bf, start=True, stop=True)
                        st = st_pool.tile([128, DIM], BF16, tag="st")
                        nc.scalar.copy(out=st, in_=pp)
                        nc.sync.dma_start(
                            out=out_d[t * 128:(t + 1) * 128, :], in_=st)

                def emit_av(pc, ppb):
                    pr = pc - 4 * g
                    for s in range(max(pr, 0), 4):
                        nc.tensor.matmul(
                            out=av[s],
                            lhsT=ppb[:, s * 128:(s + 1) * 128],
                            rhs=vo[:, pc, v_lo:v_lo + 65],
                            start=(pc == 0), stop=(pc == 4 * g + s))
                        if pc == 4 * g + s:
                            emit_norm(s)

                for c in range(nch):
                    r = c - 4 * g
                    lo = 128 * r if r > 0 else 0
                    dp = dt_pool.tile([128, 512], F32, tag="dots", name="dp")
                    nc.tensor.matmul(
                        out=dp[:, lo:512],
                        lhsT=kTh[:, c * 128:(c + 1) * 128],
                        rhs=qTh[:, g * 512 + lo:(g + 1) * 512],
                        start=True, stop=True)
                    pb = pr_pool.tile([128, 512], BF16, tag="probs", name="pb")
                    nc.scalar.activation(out=pb[:, lo:512], in_=dp[:, lo:512],
                                         func=Exp, scale=SCALE)
                    if r >= 0:
                        nc.gpsimd.tensor_mul(
                            pb[:, lo:lo + 128], pb[:, lo:lo + 128], tri)
                    pend.append((c, pb))
                    if len(pend) > 2:
                        emit_av(*pend.pop(0))
                    for _ in range(per):
                        if spread:
                            spread.pop(0)()
                for pc, ppb in pend:
                    emit_av(pc, ppb)
                for op in spread:
                    op()

            def ohT_flush(g, oh_g):
                """Transpose the block's stacked head outputs into ohT2."""
                for s in range(4):
                    pt = pj_pool.tile([128, 128], BF16, tag="pj", name="pt")
                    nc.tensor.transpose(
                        out=pt, in_=oh_g[:, s, :], identity=id_b)
                    t = g * 4 + s
                    nc.vector.tensor_copy(
                        out=ohT2[:, t * 128:(t + 1) * 128], in_=pt)

            # --- software-pipelined emission ---
            x0 = xT_ops(0)
            for op in x0[:6]:     # first DMA + transposes ahead of weight DMAs
                op()
            emit_weight_dmas()
            for op in x0[6:]:
                op()
            for op in qkv_ops(0):
                op()
            emit_wo_dma()
            for g in range(NB):
                oh_g = sm_pool.tile([128, 4, 128], BF16, tag="ohg", name="ohg",
                                    bufs=2)
                sp0 = outproj_ops(g - 1) if g > 0 else []
                emit_attn(0, g, oh_g, spread=sp0)
                last = g == NB - 1
                nxt = [] if last else xT_ops(g + 1) + qkv_ops(g + 1)
                emit_attn(1, g, oh_g, spread=nxt, tail=last)
                if not last:
                    ohT_flush(g, oh_g)
    nc.compile()
    return nc


def _get_nc():
    if "nc" not in _cache:
        _cache["nc"] = _build()
    return _cache["nc"]


def _in_maps(x, w_qkv, w_out):
    maps = []
    for c in range(NCORES):
        b = c // 4
        h0 = 2 * (c % 4)
        cols = slice(h0 * DH, (h0 + 2) * DH)  # 128 contiguous head cols
        maps.append({
            "x": np.ascontiguousarray(x[b]),
            "wq": np.ascontiguousarray(w_qkv[:, 0:512][:, cols]),
            "wk": np.ascontiguousarray(w_qkv[:, 512:1024][:, cols]),
            "wv": np.ascontiguousarray(w_qkv[:, 1024:1536][:, cols]),
            "wo": np.ascontiguousarray(w_out[cols, :]),
        })
    return maps


def _combine(results, b_out):
    out = np.zeros((B, N, DIM), np.float32)
    for c in range(NCORES):
        out[c // 4] += np.asarray(results[c]["out"], dtype=np.float32)
    out += b_out.astype(np.float32)
    return out


def kernel(**inputs):
    x = np.asarray(inputs["x"], dtype=np.float32)
    w_qkv = np.asarray(inputs["w_qkv"], dtype=np.float32)
    w_out = np.asarray(inputs["w_out"], dtype=np.float32)
    b_out = np.asarray(inputs["b_out"], dtype=np.float32)
    # inputs["mask"] is all-ones per the problem spec (key padding no-op).
    from concourse.bass_utils import run_bass_kernel_spmd
    nc = _get_nc()
    res = run_bass_kernel_spmd(nc, _in_maps(x, w_qkv, w_out), list(range(NCORES)))
    return _combine(res.results, b_out)


# revision 23
# speedup vs baseline: 2695.5581x; 2450.2645x over previous
"""Causal multi-head attention (qkv proj + attention + out proj) on 8 TRN2 cores.

Problem: x[2,2048,512] -> qkv proj (w_qkv [512,1536]) -> 8 heads x 64 dim causal
attention -> out proj (w_out [512,512] + b_out). Key-padding mask is all-ones
per the problem spec, so only the causal mask is applied.

Sharding: data-parallel over batch (2) x tensor-parallel over heads (4 groups
of 2 heads).  Core c handles batch c//4 and heads {2*(c%4), 2*(c%4)+1}.  Each
core computes its 2 heads' partial out-projection [N, DIM]; the host sums the
4 partials per batch and adds b_out (the unshard step for TP-partial outputs).

Per-core kernel:
  - Both heads processed as one 128-wide unit wherever possible: qkv
    projections produce qT2/kT2/vT2 [128, N] (heads stacked on partitions,
    M=128 matmuls), and the out-projection contracts K=128 over both heads in
    a single matmul per row tile.
  - Attention per head uses partition-base-offset slices of qT2/kT2 in
    fp32r (full rate at free dim >= 256), transposed-probs orientation:
    dotsT[j,i] = k_j . q_i per (i-block 512, j-chunk 128); exp on ScalarE with
    no max subtraction (logits bounded, softmax shift-invariant); causal mask
    multiplies only the 128x128 diagonal sub-block by a fixed triangular
    mask; P@V appends a shared ones-column to V so row sums land in PSUM free;
    normalization via fast approximate reciprocal.
  - Emission is one software-pipelined stream: attention chunks carry the
    next block's DMA/transpose/projection ops and the previous block's
    out-projection as spread filler; P@V lags its dots by one chunk so the
    TensorE never waits on ScalarE's exp.
"""

import numpy as np

B, N, DIM = 2, 2048, 512
HEADS, DH = 8, 64
SCALE = DH ** -0.5
NT = N // 128      # 16 row tiles
NB = N // 512      # 4 blocks
CC = DIM // 128    # 4 contraction chunks
NCORES = 8

_cache = {}


def _build():
    import concourse.bass as bass
    import concourse.mybir as mybir
    import concourse.tile as tile
    from concourse import bacc
    from contextlib import ExitStack

    F32 = mybir.dt.float32
    F32R = mybir.dt.float32r
    BF16 = mybir.dt.bfloat16
    Exp = mybir.ActivationFunctionType.Exp

    nc = bacc.Bacc()
    x_d = nc.declare_dram_parameter("x", [N, DIM], F32, isOutput=False).ap()
    # qkv weights feed fp32r matmuls -> declare fp32r so every producer in the
    # chain is a legal fp32r source for the BIR verifier.
    wq_d = nc.declare_dram_parameter("wq", [DIM, 128], F32R, isOutput=False).ap()
    wk_d = nc.declare_dram_parameter("wk", [DIM, 128], F32R, isOutput=False).ap()
    wv_d = nc.declare_dram_parameter("wv", [DIM, 128], F32R, isOutput=False).ap()
    wo_d = nc.declare_dram_parameter("wo", [128, DIM], F32, isOutput=False).ap()
    out_d = nc.declare_dram_parameter("out", [N, DIM], BF16, isOutput=True).ap()

    with tile.TileContext(nc) as tc:
        with ExitStack() as ctx:
            persist = ctx.enter_context(tc.tile_pool(name="persist", bufs=1))

            # --- constants ---
            id_f = persist.tile([128, 128], F32, tag="idf")
            nc.vector.memset(id_f, 0.0)
            nc.gpsimd.affine_select(
                out=id_f, in_=id_f, compare_op=mybir.AluOpType.not_equal,
                fill=1.0, base=0, pattern=[[-1, 128]], channel_multiplier=1)
            warm_sb = persist.tile([128, 128], F32, tag="warm")
            id_b = persist.tile([128, 128], BF16, tag="idb")
            nc.vector.memset(id_b, 0.0)
            nc.gpsimd.affine_select(
                out=id_b, in_=id_b, compare_op=mybir.AluOpType.not_equal,
                fill=1.0, base=0, pattern=[[-1, 128]], channel_multiplier=1)
            # tri[p, x] = 1.0 if x >= p else 0.0 (keep i >= j on the diagonal)
            tri = persist.tile([128, 128], BF16, tag="tri")
            nc.vector.memset(tri, 1.0)
            nc.gpsimd.affine_select(
                out=tri, in_=tri, compare_op=mybir.AluOpType.is_ge,
                fill=0.0, base=0, pattern=[[1, 128]], channel_multiplier=-1)

            # --- weights (DMAs emitted after block-0 x loads; see below) ---
            wq_sb = persist.tile([128, CC, 128], F32R, tag="wq")
            wk_sb = persist.tile([128, CC, 128], F32R, tag="wk")
            wv_sb = persist.tile([128, CC, 128], F32R, tag="wv")
            wo_sb = persist.tile([128, DIM], F32, tag="wo32")
            wo_bf = persist.tile([128, DIM], BF16, tag="wobf")

            def emit_weight_dmas():
                nc.sync.dma_start(
                    out=wq_sb, in_=wq_d.rearrange("(c p) d -> p c d", p=128))
                nc.sync.dma_start(
                    out=wk_sb, in_=wk_d.rearrange("(c p) d -> p c d", p=128))
                nc.sync.dma_start(
                    out=wv_sb, in_=wv_d.rearrange("(c p) d -> p c d", p=128))

            def emit_wo_dma():
                nc.sync.dma_start(out=wo_sb, in_=wo_d)
                nc.vector.tensor_copy(out=wo_bf, in_=wo_sb)

            # --- persistent activations (both heads stacked) ---
            xT = persist.tile([128, CC, N], F32R, tag="xT")
            qT2 = persist.tile([128, N], F32R, tag="qT2")
            kT2 = persist.tile([128, N], F32R, tag="kT2")
            # vo: [v_h0 (0:64) | ones (64) | v_h1 (65:129)] -- ones shared.
            # av rhs for h0 = vo[:, t, 0:65] (sum in col 64); for h1 =
            # vo[:, t, 64:129] (sum in col 0).
            vo = persist.tile([128, NT, 129], BF16, tag="vo")
            nc.vector.memset(vo, 1.0)
            ohT2 = persist.tile([128, N], BF16, tag="ohT2")

            pools = [
                tc.tile_pool(name="xs", bufs=6),
                tc.tile_pool(name="vts", bufs=2),
                tc.tile_pool(name="probs", bufs=10),
                tc.tile_pool(name="small", bufs=8),
                tc.tile_pool(name="stage", bufs=3),
                tc.tile_pool(name="proj", bufs=2, space="PSUM"),   # transposes+qkv
                tc.tile_pool(name="pdots", bufs=2, space="PSUM"),  # dots + outproj
                tc.tile_pool(name="pav", bufs=1, space="PSUM"),    # 4 tagged av banks
            ]
            (xs_pool, vt_pool, pr_pool, sm_pool, st_pool,
             pj_pool, dt_pool, av_pool) = [
                ctx.enter_context(p) for p in pools]

            # PE consumes id_f early so x transposes only wait on their DMA.
            pwarm = pj_pool.tile([128, 128], F32, tag="pj", name="pwarm")
            nc.tensor.transpose(out=pwarm, in_=id_f, identity=id_f)
            nc.vector.tensor_copy(out=warm_sb, in_=pwarm)

            def xT_ops(g):
                """Closures loading + transposing x block g into xT."""
                ops = []
                state = {}

                def mk_t(t):
                    def dma():
                        xs = xs_pool.tile([128, DIM], F32, tag="xs")
                        nc.sync.dma_start(
                            out=xs, in_=x_d[t * 128:(t + 1) * 128, :])
                        state[t] = xs

                    def mk_tr(c):
                        def f():
                            if c == 0:
                                state[(t, "px")] = pj_pool.tile(
                                    [128, 4, 128], F32, tag="pj", name="px")
                            nc.tensor.transpose(
                                out=state[(t, "px")][:, c, :],
                                in_=state[t][:, c * 128:(c + 1) * 128],
                                identity=id_f)
                        return f

                    def cp():
                        nc.vector.tensor_copy(
                            out=xT[:, :, t * 128:(t + 1) * 128],
                            in_=state.pop((t, "px")))
                        state.pop(t)
                    return [dma] + [mk_tr(c) for c in range(CC)] + [cp]

                for t in range(4 * g, 4 * g + 4):
                    ops.extend(mk_t(t))
                return ops

            def qkv_ops(g):
                """Closures projecting q/k/v (both heads at once) for block g."""
                ops = []
                state = {}

                def mk_mm(key, wsb, c):
                    def f():
                        if c == 0:
                            state[key] = pj_pool.tile(
                                [128, 512], F32, tag="pj", name=f"ps_{key}")
                        nc.tensor.matmul(
                            out=state[key],
                            lhsT=wsb[:, c, :],
                            rhs=xT[:, c, g * 512:(g + 1) * 512],
                            start=(c == 0), stop=(c == CC - 1))
                    return f

                def mk_cp(key, dst):
                    def f():
                        nc.vector.tensor_copy(
                            out=dst[:, g * 512:(g + 1) * 512],
                            in_=state.pop(key))
                    return f

                for key, (wsb, dst) in enumerate(
                        ((wq_sb, qT2), (wk_sb, kT2))):
                    for c in range(CC):
                        ops.append(mk_mm(key, wsb, c))
                    ops.append(mk_cp(key, dst))
                for c in range(CC):
                    ops.append(mk_mm("v", wv_sb, c))

                def cp_v():
                    vts = vt_pool.tile([128, 512], F32, tag="vts")
                    nc.vector.tensor_copy(out=vts, in_=state.pop("v"))
                    state["vts"] = vts
                ops.append(cp_v)

                def mk_tr(i):
                    def f():
                        if i == 0:
                            state["pv"] = pj_pool.tile(
                                [128, 4, 128], F32, tag="pj", name="pv")
                        nc.tensor.transpose(
                            out=state["pv"][:, i, :],
                            in_=state["vts"][:, i * 128:(i + 1) * 128],
                            identity=id_f)
                    return f
                for i in range(4):
                    ops.append(mk_tr(i))

                def cp_vo0():
                    nc.vector.tensor_copy(
                        out=vo[:, 4 * g:4 * g + 4, 0:64],
                        in_=state["pv"][:, :, 0:64])

                def cp_vo1():
                    nc.vector.tensor_copy(
                        out=vo[:, 4 * g:4 * g + 4, 65:129],
                        in_=state.pop("pv")[:, :, 64:128])
                    state.pop("vts", None)
                ops.extend([cp_vo0, cp_vo1])
                return ops

            def outproj_ops(g):
                """Closures for the block-g out-projection (heads fused, K=128)."""
                ops = []
                state = {}

                def mk(s):
                    t = g * 4 + s

                    def mm():
                        state[s] = pj_pool.tile(
                            [128, DIM], F32, tag="pj", name="pp")
                        nc.tensor.matmul(
                            out=state[s], lhsT=ohT2[:, t * 128:(t + 1) * 128],
                            rhs=wo_bf, start=True, stop=True)

                    def cp():
                        st = st_pool.tile([128, DIM], BF16, tag="st")
                        nc.vector.tensor_copy(out=st, in_=state.pop(s))
                        nc.sync.dma_start(
                            out=out_d[t * 128:(t + 1) * 128, :], in_=st)
                    return [mm, cp]

                for s in range(4):
                    ops.extend(mk(s))
                return ops

            def emit_attn(h, g, oh_g, spread=(), tail=False):
                """Attention for head h over i-block g; fills oh_g columns."""
                spread = list(spread)
                hb = h * 64
                qTh = qT2[hb:hb + 64, :]
                kTh = kT2[hb:hb + 64, :]
                sum_col = 64 if h == 0 else 0
                v_lo = 0 if h == 0 else 64
                av = [av_pool.tile([128, 65], F32, tag=f"av{s}", name=f"av{s}")
                      for s in range(4)]
                nch = 4 * g + 4
                per = -(-(2 * len(spread)) // nch) if spread else 0
                pend = []  # (chunk index, probs tile) whose P@V is deferred

                def emit_norm(sb):
                    rec = sm_pool.tile([128, 1], F32, tag="rec", name="rec")
                    nc.vector.reciprocal_approx_fast(
                        out=rec, in_=av[sb][:, sum_col:sum_col + 1])
                    osl = av[sb][:, 0:64] if h == 0 else av[sb][:, 1:65]
                    nc.vector.tensor_scalar_mul(
                        oh_g[:, sb, hb:hb + 64], osl, rec)
                    if tail:
                        t = g * 4 + sb
                        pt = pj_pool.tile([128, 128], BF16, tag="pj", name="pt")
                        nc.tensor.transpose(
                            out=pt, in_=oh_g[:, sb, :], identity=id_b)
                        nc.vector.tensor_copy(
                            out=ohT2[:, t * 128:(t + 1) * 128], in_=pt)
                        pp = pj_pool.tile([128, DIM], F32, tag="pj", name="pp")
                        nc.tensor.matmul(
                            out=pp, lhsT=ohT2[:, t * 128:(t + 1) * 128],
                            rhs=wo_bf, start=True, stop=True)
                        st = st_pool.tile([128, DIM], BF16, tag="st")
                        nc.scalar.copy(out=st, in_=pp)
                        nc.sync.dma_start(
                            out=out_d[t * 128:(t + 1) * 128, :], in_=st)

                def emit_av(pc, ppb):
                    pr = pc - 4 * g
                    for s in range(max(pr, 0), 4):
                        nc.tensor.matmul(
                            out=av[s],
                            lhsT=ppb[:, s * 128:(s + 1) * 128],
                            rhs=vo[:, pc, v_lo:v_lo + 65],
                            start=(pc == 0), stop=(pc == 4 * g + s))
                        if pc == 4 * g + s:
                            emit_norm(s)

                for c in range(nch):
                    r = c - 4 * g
                    lo = 128 * r if r > 0 else 0
                    dp = dt_pool.tile([128, 512], F32, tag="dots", name="dp")
                    nc.tensor.matmul(
                        out=dp[:, lo:512],
                        lhsT=kTh[:, c * 128:(c + 1) * 128],
                        rhs=qTh[:, g * 512 + lo:(g + 1) * 512],
                        start=True, stop=True)
                    pb = pr_pool.tile([128, 512], BF16, tag="probs", name="pb")
                    nc.scalar.activation(out=pb[:, lo:512], in_=dp[:, lo:512],
                                         func=Exp, scale=SCALE)
                    if r >= 0:
                        nc.gpsimd.tensor_mul(
                            pb[:, lo:lo + 128], pb[:, lo:lo + 128], tri)
                    pend.append((c, pb))
                    if len(pend) > 3:
                        emit_av(*pend.pop(0))
                    for _ in range(per):
                        if spread:
                            spread.pop(0)()
                for pc, ppb in pend:
                    emit_av(pc, ppb)
                for op in spread:
                    op()

            def ohT_flush(g, oh_g):
                """Transpose the block's stacked head outputs into ohT2."""
                for s in range(4):
                    pt = pj_pool.tile([128, 128], BF16, tag="pj", name="pt")
                    nc.tensor.transpose(
                        out=pt, in_=oh_g[:, s, :], identity=id_b)
                    t = g * 4 + s
                    nc.vector.tensor_copy(
                        out=ohT2[:, t * 128:(t + 1) * 128], in_=pt)

            # --- software-pipelined emission ---
            x0 = xT_ops(0)
            for op in x0[:6]:     # first DMA + transposes ahead of weight DMAs
                op()
            emit_weight_dmas()
            for op in x0[6:]:
                op()
            for op in qkv_ops(0):
                op()
            emit_wo_dma()
            for g in range(NB):
                oh_g = sm_pool.tile([128, 4, 128], BF16, tag="ohg", name="ohg",
                                    bufs=2)
                sp0 = outproj_ops(g - 1) if g > 0 else []
                emit_attn(0, g, oh_g, spread=sp0)
                last = g == NB - 1
                nxt = [] if last else xT_ops(g + 1) + qkv_ops(g + 1)
                emit_attn(1, g, oh_g, spread=nxt, tail=last)
                if not last:
                    ohT_flush(g, oh_g)
    nc.compile()
    return nc


def _get_nc():
    if "nc" not in _cache:
        _cache["nc"] = _build()
    return _cache["nc"]


def _in_maps(x, w_qkv, w_out):
    maps = []
    for c in range(NCORES):
        b = c // 4
        h0 = 2 * (c % 4)
        cols = slice(h0 * DH, (h0 + 2) * DH)  # 128 contiguous head cols
        maps.append({
            "x": np.ascontiguousarray(x[b]),
            "wq": np.ascontiguousarray(w_qkv[:, 0:512][:, cols]),
            "wk": np.ascontiguousarray(w_qkv[:, 512:1024][:, cols]),
            "wv": np.ascontiguousarray(w_qkv[:, 1024:1536][:, cols]),
            "wo": np.ascontiguousarray(w_out[cols, :]),
        })
    return maps


def _combine(results, b_out):
    out = np.zeros((B, N, DIM), np.float32)
    for c in range(NCORES):
        out[c // 4] += np.asarray(results[c]["out"], dtype=np.float32)
    out += b_out.astype(np.float32)
    return out


def kernel(**inputs):
    x = np.asarray(inputs["x"], dtype=np.float32)
    w_qkv = np.asarray(inputs["w_qkv"], dtype=np.float32)
    w_out = np.asarray(inputs["w_out"], dtype=np.float32)
    b_out = np.asarray(inputs["b_out"], dtype=np.float32)
    # inputs["mask"] is all-ones per the problem spec (key padding no-op).
    from concourse.bass_utils import run_bass_kernel_spmd
    nc = _get_nc()
    res = run_bass_kernel_spmd(nc, _in_maps(x, w_qkv, w_out), list(range(NCORES)))
    return _combine(res.results, b_out)


# revision 24
# speedup vs baseline: 2745.1030x; 1.0184x over previous
"""Causal multi-head attention (qkv proj + attention + out proj) on 8 TRN2 cores.

Problem: x[2,2048,512] -> qkv proj (w_qkv [512,1536]) -> 8 heads x 64 dim causal
attention -> out proj (w_out [512,512] + b_out). Key-padding mask is all-ones
per the problem spec, so only the causal mask is applied.

Sharding: data-parallel over batch (2) x tensor-parallel over heads (4 groups
of 2 heads).  Core c handles batch c//4 and heads {2*(c%4), 2*(c%4)+1}.  Each
core computes its 2 heads' partial out-projection [N, DIM]; the host sums the
4 partials per batch and adds b_out (the unshard step for TP-partial outputs).

Per-core kernel:
  - Both heads processed as one 128-wide unit wherever possible: qkv
    projections produce qT2/kT2/vT2 [128, N] (heads stacked on partitions,
    M=128 matmuls), and the out-projection contracts K=128 over both heads in
    a single matmul per row tile.
  - Attention per head uses partition-base-offset slices of qT2/kT2 in
    fp32r (full rate at free dim >= 256), transposed-probs orientation:
    dotsT[j,i] = k_j . q_i per (i-block 512, j-chunk 128); exp on ScalarE with
    no max subtraction (logits bounded, softmax shift-invariant); causal mask
    multiplies only the 128x128 diagonal sub-block by a fixed triangular
    mask; P@V appends a shared ones-column to V so row sums land in PSUM free;
    normalization via fast approximate reciprocal.
  - Emission is one software-pipelined stream: attention chunks carry the
    next block's DMA/transpose/projection ops and the previous block's
    out-projection as spread filler; P@V lags its dots by one chunk so the
    TensorE never waits on ScalarE's exp.
"""

import numpy as np

B, N, DIM = 2, 2048, 512
HEADS, DH = 8, 64
SCALE = DH ** -0.5
NT = N // 128      # 16 row tiles
NB = N // 512      # 4 blocks
CC = DIM // 128    # 4 contraction chunks
NCORES = 8

_cache = {}


def _build():
    import concourse.bass as bass
    import concourse.mybir as mybir
    import concourse.tile as tile
    from concourse import bacc
    from contextlib import ExitStack

    F32 = mybir.dt.float32
    F32R = mybir.dt.float32r
    BF16 = mybir.dt.bfloat16
    Exp = mybir.ActivationFunctionType.Exp

    nc = bacc.Bacc()
    x_d = nc.declare_dram_parameter("x", [N, DIM], F32, isOutput=False).ap()
    # qkv weights feed fp32r matmuls -> declare fp32r so every producer in the
    # chain is a legal fp32r source for the BIR verifier.
    wq_d = nc.declare_dram_parameter("wq", [DIM, 128], F32R, isOutput=False).ap()
    wk_d = nc.declare_dram_parameter("wk", [DIM, 128], F32R, isOutput=False).ap()
    wv_d = nc.declare_dram_parameter("wv", [DIM, 128], F32R, isOutput=False).ap()
    wo_d = nc.declare_dram_parameter("wo", [128, DIM], F32, isOutput=False).ap()
    out_d = nc.declare_dram_parameter("out", [N, DIM], BF16, isOutput=True).ap()

    with tile.TileContext(nc) as tc:
        with ExitStack() as ctx:
            persist = ctx.enter_context(tc.tile_pool(name="persist", bufs=1))

            # --- constants ---
            id_f = persist.tile([128, 128], F32, tag="idf")
            nc.vector.memset(id_f, 0.0)
            nc.gpsimd.affine_select(
                out=id_f, in_=id_f, compare_op=mybir.AluOpType.not_equal,
                fill=1.0, base=0, pattern=[[-1, 128]], channel_multiplier=1)
            warm_sb = persist.tile([128, 128], F32, tag="warm")
            id_b = persist.tile([128, 128], BF16, tag="idb")
            nc.vector.memset(id_b, 0.0)
            nc.gpsimd.affine_select(
                out=id_b, in_=id_b, compare_op=mybir.AluOpType.not_equal,
                fill=1.0, base=0, pattern=[[-1, 128]], channel_multiplier=1)
            # tri[p, x] = 1.0 if x >= p else 0.0 (keep i >= j on the diagonal)
            tri = persist.tile([128, 128], BF16, tag="tri")
            nc.vector.memset(tri, 1.0)
            nc.gpsimd.affine_select(
                out=tri, in_=tri, compare_op=mybir.AluOpType.is_ge,
                fill=0.0, base=0, pattern=[[1, 128]], channel_multiplier=-1)

            # --- weights (DMAs emitted after block-0 x loads; see below) ---
            wq_sb = persist.tile([128, CC, 128], F32R, tag="wq")
            wk_sb = persist.tile([128, CC, 128], F32R, tag="wk")
            wv_sb = persist.tile([128, CC, 128], F32R, tag="wv")
            wo_sb = persist.tile([128, DIM], F32, tag="wo32")
            wo_bf = persist.tile([128, DIM], BF16, tag="wobf")

            def emit_weight_dmas():
                nc.sync.dma_start(
                    out=wq_sb, in_=wq_d.rearrange("(c p) d -> p c d", p=128))
                nc.sync.dma_start(
                    out=wk_sb, in_=wk_d.rearrange("(c p) d -> p c d", p=128))
                nc.sync.dma_start(
                    out=wv_sb, in_=wv_d.rearrange("(c p) d -> p c d", p=128))

            def emit_wo_dma():
                nc.sync.dma_start(out=wo_sb, in_=wo_d)
                nc.vector.tensor_copy(out=wo_bf, in_=wo_sb)

            # --- persistent activations (both heads stacked) ---
            xT = persist.tile([128, CC, N], F32R, tag="xT")
            qT2 = persist.tile([128, N], F32R, tag="qT2")
            kT2 = persist.tile([128, N], F32R, tag="kT2")
            # vo: [v_h0 (0:64) | ones (64) | v_h1 (65:129)] -- ones shared.
            # av rhs for h0 = vo[:, t, 0:65] (sum in col 64); for h1 =
            # vo[:, t, 64:129] (sum in col 0).
            vo = persist.tile([128, NT, 129], BF16, tag="vo")
            nc.vector.memset(vo, 1.0)
            ohT2 = persist.tile([128, N], BF16, tag="ohT2")

            pools = [
                tc.tile_pool(name="xs", bufs=6),
                tc.tile_pool(name="vts", bufs=3),
                tc.tile_pool(name="probs", bufs=12),
                tc.tile_pool(name="small", bufs=8),
                tc.tile_pool(name="stage", bufs=4),
                tc.tile_pool(name="proj", bufs=2, space="PSUM"),   # transposes+qkv
                tc.tile_pool(name="pdots", bufs=2, space="PSUM"),  # dots + outproj
                tc.tile_pool(name="pav", bufs=1, space="PSUM"),    # 4 tagged av banks
            ]
            (xs_pool, vt_pool, pr_pool, sm_pool, st_pool,
             pj_pool, dt_pool, av_pool) = [
                ctx.enter_context(p) for p in pools]

            # PE consumes id_f early so x transposes only wait on their DMA.
            pwarm = pj_pool.tile([128, 128], F32, tag="pj", name="pwarm")
            nc.tensor.transpose(out=pwarm, in_=id_f, identity=id_f)
            nc.vector.tensor_copy(out=warm_sb, in_=pwarm)

            def xT_ops(g):
                """Closures loading + transposing x block g into xT."""
                ops = []
                state = {}

                def mk_t(t):
                    def dma():
                        xs = xs_pool.tile([128, DIM], F32, tag="xs")
                        nc.sync.dma_start(
                            out=xs, in_=x_d[t * 128:(t + 1) * 128, :])
                        state[t] = xs

                    def mk_tr(c):
                        def f():
                            if c == 0:
                                state[(t, "px")] = pj_pool.tile(
                                    [128, 4, 128], F32, tag="pj", name="px")
                            nc.tensor.transpose(
                                out=state[(t, "px")][:, c, :],
                                in_=state[t][:, c * 128:(c + 1) * 128],
                                identity=id_f)
                        return f

                    def cp():
                        nc.vector.tensor_copy(
                            out=xT[:, :, t * 128:(t + 1) * 128],
                            in_=state.pop((t, "px")))
                        state.pop(t)
                    return [dma] + [mk_tr(c) for c in range(CC)] + [cp]

                for t in range(4 * g, 4 * g + 4):
                    ops.extend(mk_t(t))
                return ops

            def qkv_ops(g):
                """Closures projecting q/k/v (both heads at once) for block g."""
                ops = []
                state = {}

                def mk_mm(key, wsb, c):
                    def f():
                        if c == 0:
                            state[key] = pj_pool.tile(
                                [128, 512], F32, tag="pj", name=f"ps_{key}")
                        nc.tensor.matmul(
                            out=state[key],
                            lhsT=wsb[:, c, :],
                            rhs=xT[:, c, g * 512:(g + 1) * 512],
                            start=(c == 0), stop=(c == CC - 1))
                    return f

                def mk_cp(key, dst):
                    def f():
                        nc.vector.tensor_copy(
                            out=dst[:, g * 512:(g + 1) * 512],
                            in_=state.pop(key))
                    return f

                for key, (wsb, dst) in enumerate(
                        ((wq_sb, qT2), (wk_sb, kT2))):
                    for c in range(CC):
                        ops.append(mk_mm(key, wsb, c))
                    ops.append(mk_cp(key, dst))
                for c in range(CC):
                    ops.append(mk_mm("v", wv_sb, c))

                def cp_v():
                    vts = vt_pool.tile([128, 512], F32, tag="vts")
                    nc.vector.tensor_copy(out=vts, in_=state.pop("v"))
                    state["vts"] = vts
                ops.append(cp_v)

                def mk_tr(i):
                    def f():
                        if i == 0:
                            state["pv"] = pj_pool.tile(
                                [128, 4, 128], F32, tag="pj", name="pv")
                        nc.tensor.transpose(
                            out=state["pv"][:, i, :],
                            in_=state["vts"][:, i * 128:(i + 1) * 128],
                            identity=id_f)
                    return f
                for i in range(4):
                    ops.append(mk_tr(i))

                def cp_vo0():
                    nc.vector.tensor_copy(
                        out=vo[:, 4 * g:4 * g + 4, 0:64],
                        in_=state["pv"][:, :, 0:64])

                def cp_vo1():
                    nc.vector.tensor_copy(
                        out=vo[:, 4 * g:4 * g + 4, 65:129],
                        in_=state.pop("pv")[:, :, 64:128])
                    state.pop("vts", None)
                ops.extend([cp_vo0, cp_vo1])
                return ops

            def outproj_ops(g):
                """Closures for the block-g out-projection (heads fused, K=128)."""
                ops = []
                state = {}

                def mk(s):
                    t = g * 4 + s

                    def mm():
                        state[s] = pj_pool.tile(
                            [128, DIM], F32, tag="pj", name="pp")
                        nc.tensor.matmul(
                            out=state[s], lhsT=ohT2[:, t * 128:(t + 1) * 128],
                            rhs=wo_bf, start=True, stop=True)

                    def cp():
                        st = st_pool.tile([128, DIM], BF16, tag="st")
                        nc.vector.tensor_copy(out=st, in_=state.pop(s))
                        nc.sync.dma_start(
                            out=out_d[t * 128:(t + 1) * 128, :], in_=st)
                    return [mm, cp]

                for s in range(4):
                    ops.extend(mk(s))
                return ops

            def emit_attn(h, g, oh_g, spread=(), tail=False):
                """Attention for head h over i-block g; fills oh_g columns."""
                spread = list(spread)
                hb = h * 64
                qTh = qT2[hb:hb + 64, :]
                kTh = kT2[hb:hb + 64, :]
                sum_col = 64 if h == 0 else 0
                v_lo = 0 if h == 0 else 64
                av = [av_pool.tile([128, 65], F32, tag=f"av{s}", name=f"av{s}")
                      for s in range(4)]
                nch = 4 * g + 4
                per = -(-(2 * len(spread)) // nch) if spread else 0
                pend = []  # (chunk index, probs tile) whose P@V is deferred

                def emit_norm(sb):
                    rec = sm_pool.tile([128, 1], F32, tag="rec", name="rec")
                    nc.vector.reciprocal_approx_fast(
                        out=rec, in_=av[sb][:, sum_col:sum_col + 1])
                    osl = av[sb][:, 0:64] if h == 0 else av[sb][:, 1:65]
                    nc.vector.tensor_scalar_mul(
                        oh_g[:, sb, hb:hb + 64], osl, rec)
                    if tail:
                        t = g * 4 + sb
                        pt = pj_pool.tile([128, 128], BF16, tag="pj", name="pt")
                        nc.tensor.transpose(
                            out=pt, in_=oh_g[:, sb, :], identity=id_b)
                        nc.vector.tensor_copy(
                            out=ohT2[:, t * 128:(t + 1) * 128], in_=pt)
                        pp = pj_pool.tile([128, DIM], F32, tag="pj", name="pp")
                        nc.tensor.matmul(
                            out=pp, lhsT=ohT2[:, t * 128:(t + 1) * 128],
                            rhs=wo_bf, start=True, stop=True)
                        st = st_pool.tile([128, DIM], BF16, tag="st")
                        nc.scalar.copy(out=st, in_=pp)
                        nc.sync.dma_start(
                            out=out_d[t * 128:(t + 1) * 128, :], in_=st)

                def emit_av(pc, ppb):
                    pr = pc - 4 * g
                    for s in range(max(pr, 0), 4):
                        nc.tensor.matmul(
                            out=av[s],
                            lhsT=ppb[:, s * 128:(s + 1) * 128],
                            rhs=vo[:, pc, v_lo:v_lo + 65],
                            start=(pc == 0), stop=(pc == 4 * g + s))
                        if pc == 4 * g + s:
                            emit_norm(s)

                for c in range(nch):
                    r = c - 4 * g
                    lo = 128 * r if r > 0 else 0
                    dp = dt_pool.tile([128, 512], F32, tag="dots", name="dp")
                    nc.tensor.matmul(
                        out=dp[:, lo:512],
                        lhsT=kTh[:, c * 128:(c + 1) * 128],
                        rhs=qTh[:, g * 512 + lo:(g + 1) * 512],
                        start=True, stop=True)
                    pb = pr_pool.tile([128, 512], BF16, tag="probs", name="pb")
                    nc.scalar.activation(out=pb[:, lo:512], in_=dp[:, lo:512],
                                         func=Exp, scale=SCALE)
                    if r >= 0:
                        nc.gpsimd.tensor_mul(
                            pb[:, lo:lo + 128], pb[:, lo:lo + 128], tri)
                    pend.append((c, pb))
                    if len(pend) > 4:
                        emit_av(*pend.pop(0))
                    for _ in range(per):
                        if spread:
                            spread.pop(0)()
                for pc, ppb in pend:
                    emit_av(pc, ppb)
                for op in spread:
                    op()

            def ohT_flush(g, oh_g):
                """Transpose the block's stacked head outputs into ohT2."""
                for s in range(4):
                    pt = pj_pool.tile([128, 128], BF16, tag="pj", name="pt")
                    nc.tensor.transpose(
                        out=pt, in_=oh_g[:, s, :], identity=id_b)
                    t = g * 4 + s
                    nc.vector.tensor_copy(
                        out=ohT2[:, t * 128:(t + 1) * 128], in_=pt)

            # --- software-pipelined emission ---
            x0 = xT_ops(0)
            for op in x0[:6]:     # first DMA + transposes ahead of weight DMAs
                op()
            emit_weight_dmas()
            for op in x0[6:]:
                op()
            for op in qkv_ops(0):
                op()
            emit_wo_dma()
            for g in range(NB):
                oh_g = sm_pool.tile([128, 4, 128], BF16, tag="ohg", name="ohg",
                                    bufs=2)
                sp0 = outproj_ops(g - 1) if g > 0 else []
                emit_attn(0, g, oh_g, spread=sp0)
                last = g == NB - 1
                nxt = [] if last else xT_ops(g + 1) + qkv_ops(g + 1)
                emit_attn(1, g, oh_g, spread=nxt, tail=last)
                if not last:
                    ohT_flush(g, oh_g)
    nc.compile()
    return nc


def _get_nc():
    if "nc" not in _cache:
        _cache["nc"] = _build()
    return _cache["nc"]


def _in_maps(x, w_qkv, w_out):
    maps = []
    for c in range(NCORES):
        b = c // 4
        h0 = 2 * (c % 4)
        cols = slice(h0 * DH, (h0 + 2) * DH)  # 128 contiguous head cols
        maps.append({
            "x": np.ascontiguousarray(x[b]),
            "wq": np.ascontiguousarray(w_qkv[:, 0:512][:, cols]),
            "wk": np.ascontiguousarray(w_qkv[:, 512:1024][:, cols]),
            "wv": np.ascontiguousarray(w_qkv[:, 1024:1536][:, cols]),
            "wo": np.ascontiguousarray(w_out[cols, :]),
        })
    return maps


def _combine(results, b_out):
    out = np.zeros((B, N, DIM), np.float32)
    for c in range(NCORES):
        out[c // 4] += np.asarray(results[c]["out"], dtype=np.float32)
    out += b_out.astype(np.float32)
    return out


def kernel(**inputs):
    x = np.asarray(inputs["x"], dtype=np.float32)
    w_qkv = np.asarray(inputs["w_qkv"], dtype=np.float32)
    w_out = np.asarray(inputs["w_out"], dtype=np.float32)
    b_out = np.asarray(inputs["b_out"], dtype=np.float32)
    # inputs["mask"] is all-ones per the problem spec (key padding no-op).
    from concourse.bass_utils import run_bass_kernel_spmd
    nc = _get_nc()
    res = run_bass_kernel_spmd(nc, _in_maps(x, w_qkv, w_out), list(range(NCORES)))
    return _combine(res.results, b_out)
